# revision 12
# baseline (speedup 1.0000x reference)
"""Dense dilated KNN graph kernel for Trainium2 (8 NeuronCores).

Problem: x [10000, 512] f32, k=9.
reference: column-L2-normalize x (dim=0) -> xn; dist_ij = ||xn_i - xn_j||^2;
edge_idx = stack((top9_idx(-dist), center_idx)) -> [2, 10000, 9] int32.

Math: row i's k-NN ranking of -dist equals the DESCENDING ranking of
score(i,j) = xn_i . xn_j - ||xn_j||^2/2  (the sq_i term is constant per row).
score(i,i) is always the row max (score_ii - score_ij = ||xn_i-xn_j||^2/2 >= 0),
so top-9 = [self] + top-8 of the rest, and k=9 maps exactly onto the DVE's
8-wide max/max_index ops after knocking out the diagonal.

Precision: fp32 matmuls cannot PSUM-accumulate on this toolchain, and a plain
bf16 matmul is far too coarse for the ~1e-5 gaps between neighbor distances.
So xn is stored as a bf16 hi/lo pair (xn ~ hi + lo, |lo| <~ 2^-9 |xn|) and
G = hi@hi.T + hi@lo.T + lo@hi.T, giving ~3e-8 absolute score error (fp32
class) at full bf16 matmul speed.

Sharding: rows split across 8 cores (1250 each). Every core receives a
ROTATED copy of x (rolled by -1250*c) so one SPMD program serves all cores:
"my" rows are local rows 0..1249 and the self-diagonal lands at
compile-time-known positions.

Per core:
  pass 1: load x, PE-transpose, ACT Square-accumulate -> column norms
  pass 2: reload x, normalize (DVE), split to bf16 hi/lo, PE-transpose into
          hi_c/lo_c [128, 10000] x 4 chunks; ACT Square-accum gives row
          norms sq_j for free; sq encoded as bf16 hi+lo rows [2, 10000]
  main:   per (row-tile 125 x col-chunk 500): 12 bf16 matmuls + sq aug-row
          matmul + diagonal-knockout matmul accumulate score into PSUM;
          ACT evacuates into score quarters [125, 2500]; DVE max/max_index
          produce top-8 per quarter -> 32 candidates/row to DRAM.
Host: merge candidates, take top-8 by (value desc, index asc), prepend self.
"""

import time

import numpy as np
import ml_dtypes

import concourse.bacc as bacc
import concourse.mybir as mybir
import concourse.tile as tile
from concourse.masks import make_identity
from concourse.bass_utils import run_bass_kernel_spmd

N = 10000
D = 512
NCORES = 8
R = N // NCORES          # 1250 rows per core
TM = 125                 # row-tile size (PSUM out partitions)
NT = R // TM             # 10 row tiles
W = 500                  # col chunk (one PSUM bank at fp32)
NCH = N // W             # 20 col chunks
QW = 2500                # score quarter width
NQ = N // QW             # 4 quarters
CPQ = QW // W            # 5 chunks per quarter
KC = D // 128            # 4 contraction chunks
G = (N + 127) // 128     # 79 row groups for load/transpose (78 full + 16)

F32 = mybir.dt.float32
BF16 = mybir.dt.bfloat16
U32 = mybir.dt.uint32
COPY = mybir.ActivationFunctionType.Copy
SQUARE = mybir.ActivationFunctionType.Square
SQRT = mybir.ActivationFunctionType.Sqrt

NEG_BIG = -1e30

_CACHE = {}


def build_nc():
    nc = bacc.Bacc("TRN2", target_bir_lowering=False, debug=False,
                   num_devices=NCORES)

    xin = nc.dram_tensor("xin", [N, D], F32, kind="ExternalInput")
    vals_out = nc.dram_tensor("vals_out", [R, 8 * NQ], F32, kind="ExternalOutput")
    idx_out = nc.dram_tensor("idx_out", [R, 8 * NQ], U32, kind="ExternalOutput")
    # DRAM scratch for layout shuffles (partition-dim <-> free-dim folds)
    dinv = nc.dram_tensor("dinv", [KC, 128], F32)
    dsq = nc.dram_tensor("dsq", [2, G * 128], BF16)

    with tile.TileContext(nc) as tc:
        with (
            tc.tile_pool(name="big", bufs=1) as big,
            tc.tile_pool(name="xt", bufs=2) as xtp,
            tc.tile_pool(name="f32s", bufs=2) as f32s,
            tc.tile_pool(name="bf16s", bufs=2) as bf16s,
            tc.tile_pool(name="outs", bufs=4) as outp,
            tc.tile_pool(name="pt", bufs=2, space="PSUM") as ptp,
            tc.tile_pool(name="prep", bufs=1, space="PSUM") as prep,
            tc.tile_pool(name="pm", bufs=5, space="PSUM") as pmp,
        ):
            # ---- constants ----
            identf = big.tile([128, 128], F32, tag="identf")
            make_identity(nc, identf)
            identb = big.tile([128, 128], BF16, tag="identb")
            nc.vector.tensor_copy(identb, identf)
            negeye = big.tile([128, 128], BF16, tag="negeye")
            nc.vector.tensor_scalar_mul(negeye, identb, NEG_BIG)
            ones2 = big.tile([2, TM], BF16, tag="ones2")
            nc.vector.memset(ones2, 1.0)
            ones_k1 = big.tile([1, 128], F32, tag="ones_k1")
            nc.vector.memset(ones_k1, 1.0)

            # ---- persistent big buffers ----
            hi = [big.tile([128, N], BF16, tag=f"hi{c}", name=f"hi{c}")
                  for c in range(KC)]
            lo = [big.tile([128, N], BF16, tag=f"lo{c}", name=f"lo{c}")
                  for c in range(KC)]
            sqh = big.tile([2, N], BF16, tag="sqh")          # -sq/2 hi, lo rows
            score = big.tile([128, QW], F32, tag="score", name="score")
            part = [big.tile([128, G], F32, tag=f"part{c}", name=f"part{c}")
                    for c in range(KC)]                      # colnorm partials
            cn = big.tile([128, KC], F32, tag="cn")
            inv = big.tile([128, KC], F32, tag="inv")
            invT = big.tile([KC, 128], F32, tag="invT")
            invrep = big.tile([128, D], F32, tag="invrep")
            sq_nat = big.tile([128, G], F32, tag="sq_nat")   # row norms, [p, g]
            nc.vector.memset(sq_nat, 0.0)  # tail of last group never written
            sq79 = big.tile([128, G], F32, tag="sq79")       # -sq/2 f32
            hi79 = big.tile([128, G], BF16, tag="hi79")
            lo79 = big.tile([128, G], BF16, tag="lo79")
            hi79f = big.tile([128, G], F32, tag="hi79f")
            sqT = big.tile([G, 128], BF16, tag="sqT")
            sqT2 = big.tile([G, 128], BF16, tag="sqT2")

            # ---- pass 1: column norms ----
            # transpose raw x blocks; ACT Square-accum along free dim (rows)
            for g in range(G):
                r0 = 128 * g
                rn = min(128, N - r0)
                xt = xtp.tile([128, D], F32, tag="xt")
                nc.sync.dma_start(xt[:rn, :], xin[r0:r0 + rn, :])
                for c in range(KC):
                    pt = ptp.tile([128, 128], F32, tag="pt")
                    nc.tensor.transpose(pt[:, :rn], xt[:rn, 128 * c:128 * (c + 1)],
                                        identf[:rn, :rn])
                    dump = f32s.tile([128, D], F32, tag="xn32", name="dump")
                    nc.scalar.activation(dump[:, :rn], pt[:, :rn], SQUARE,
                                         accum_out=part[c][:, g:g + 1])

            # finalize column norms -> inv = 1/max(sqrt(sum), eps)
            for c in range(KC):
                nc.vector.tensor_reduce(cn[:, c:c + 1], part[c],
                                        axis=mybir.AxisListType.X,
                                        op=mybir.AluOpType.add)
            nc.scalar.activation(cn, cn, SQRT)
            nc.vector.tensor_scalar_max(cn, cn, 1e-12)
            nc.vector.reciprocal(inv, cn)

            # replicate inv over partitions in natural layout:
            # inv [128,4] -T-> invT [4,128] -DRAM-> flat row -> K=1 matmul bcast
            pti = ptp.tile([KC, 128], F32, tag="pt")
            nc.tensor.transpose(pti, inv, identf)
            nc.scalar.activation(invT, pti, COPY)
            nc.sync.dma_start(dinv[:], invT)
            invrow = score[0:1, 0:D]         # score buf is free in the prologue
            nc.sync.dma_start(invrow, dinv.ap().rearrange("a b -> (a b)")[None, :])
            pri = prep.tile([128, D], F32, tag="prep")
            nc.tensor.matmul(pri, ones_k1, invrow, start=True, stop=True)
            nc.scalar.activation(invrep, pri, COPY)

            # ---- pass 2: normalize, split hi/lo, transpose; row norms ----
            for g in range(G):
                r0 = 128 * g
                rn = min(128, N - r0)
                xt = xtp.tile([128, D], F32, tag="xt")
                nc.sync.dma_start(xt[:rn, :], xin[r0:r0 + rn, :])
                xn32 = f32s.tile([128, D], F32, tag="xn32")
                nc.vector.tensor_mul(xn32[:rn, :], xt[:rn, :], invrep[:rn, :])
                hin = bf16s.tile([128, D], BF16, tag="hin")
                nc.vector.tensor_copy(hin[:rn, :], xn32[:rn, :])
                # raw x in xt is dead after the mul above -- reuse as f32 scratch
                nc.vector.tensor_copy(xt[:rn, :], hin[:rn, :])
                lon = bf16s.tile([128, D], BF16, tag="lon")
                nc.vector.tensor_sub(lon[:rn, :], xn32[:rn, :], xt[:rn, :])
                # row-norm contribution (free-dim accum over all 512 d);
                # xt is dead again -- reuse it as the ACT dump
                nc.scalar.activation(xt[:rn, :], xn32[:rn, :], SQUARE,
                                     accum_out=sq_nat[:rn, g:g + 1])
                for c in range(KC):
                    cs = slice(128 * c, 128 * (c + 1))
                    pt = ptp.tile([128, 128], BF16, tag="pt", name="ptb")
                    nc.tensor.transpose(pt[:, :rn], hin[:rn, cs], identb[:rn, :rn])
                    nc.scalar.activation(hi[c][:, r0:r0 + rn], pt[:, :rn], COPY)
                    pt2 = ptp.tile([128, 128], BF16, tag="pt", name="ptb2")
                    nc.tensor.transpose(pt2[:, :rn], lon[:rn, cs], identb[:rn, :rn])
                    nc.scalar.activation(lo[c][:, r0:r0 + rn], pt2[:, :rn], COPY)

            # ---- sq rows: -sq/2 as bf16 hi+lo in [2, N] layout ----
            nc.vector.tensor_scalar_mul(sq79, sq_nat, -0.5)
            nc.vector.tensor_scalar_mul(hi79, sq_nat, -0.5)   # cast to bf16
            nc.vector.tensor_copy(hi79f, hi79)
            nc.vector.tensor_sub(lo79, sq79, hi79f)
            ptq = ptp.tile([G, 128], BF16, tag="pt", name="ptq")
            nc.tensor.transpose(ptq, hi79, identb)
            nc.scalar.activation(sqT, ptq, COPY)
            ptq2 = ptp.tile([G, 128], BF16, tag="pt", name="ptq2")
            nc.tensor.transpose(ptq2, lo79, identb)
            nc.scalar.activation(sqT2, ptq2, COPY)
            nc.sync.dma_start(dsq[0:1, :].rearrange("a (g r) -> (a g) r", g=G), sqT)
            nc.sync.dma_start(dsq[1:2, :].rearrange("a (g r) -> (a g) r", g=G), sqT2)
            nc.sync.dma_start(sqh[0:1, :], dsq[0:1, 0:N])
            nc.sync.dma_start(sqh[1:2, :], dsq[1:2, 0:N])

            # ---- main loop ----
            for t in range(NT):
                rbase = TM * t
                diag_ch = rbase // W
                diag_off = rbase - W * diag_ch
                for q in range(NQ):
                    sc = score
                    for j in range(CPQ):
                        n = CPQ * q + j
                        ns = slice(W * n, W * (n + 1))
                        rs = slice(rbase, rbase + TM)
                        pm = pmp.tile([TM, W], F32, tag="pm")
                        for c in range(KC):
                            nc.tensor.matmul(pm, hi[c][:, rs], hi[c][:, ns],
                                             start=(c == 0), stop=False)
                            nc.tensor.matmul(pm, hi[c][:, rs], lo[c][:, ns],
                                             start=False, stop=False)
                            nc.tensor.matmul(pm, lo[c][:, rs], hi[c][:, ns],
                                             start=False, stop=False)
                        is_diag = (n == diag_ch)
                        nc.tensor.matmul(pm, ones2, sqh[:, ns],
                                         start=False, stop=not is_diag)
                        if is_diag:
                            nc.tensor.matmul(
                                pm[:, diag_off:diag_off + TM],
                                identb[:TM, :TM], negeye[:TM, :TM],
                                start=False, stop=True)
                        nc.scalar.activation(sc[:TM, W * j:W * (j + 1)], pm, COPY)
                    mval = outp.tile([TM, 8], F32, tag="mv")
                    midx = outp.tile([TM, 8], U32, tag="mi")
                    nc.vector.max(out=mval, in_=sc[:TM, :])
                    nc.vector.max_index(out=midx, in_max=mval, in_values=sc[:TM, :])
                    nc.sync.dma_start(vals_out[rs, 8 * q:8 * (q + 1)], mval)
                    nc.sync.dma_start(idx_out[rs, 8 * q:8 * (q + 1)], midx)

    nc.compile()
    return nc


def _get_nc():
    if "nc" not in _CACHE:
        _CACHE["nc"] = build_nc()
    return _CACHE["nc"]


def _recompute_rows_f64(x, rows):
    """Exact f64 top-8 (excluding self) for the given rows."""
    xf = x.astype(np.float64)
    cnorm = np.sqrt((xf * xf).sum(0, keepdims=True))
    xn = xf / np.maximum(cnorm, 1e-12)
    sq = (xn * xn).sum(1)
    out = {}
    for r in rows:
        s = xn[r] @ xn.T - sq / 2.0
        s[r] = -np.inf
        idx = np.lexsort((np.arange(N), -s))[:8]
        out[r] = idx
    return out


def _merge(vals, idxs, x):
    """vals/idxs: [NCORES, R, 32] device candidates -> nn_idx [N, 9] int32."""
    q_off = (np.arange(8 * NQ, dtype=np.int64) // 8) * QW
    j_loc = idxs.astype(np.int64) + q_off[None, None, :]
    j_glob = (j_loc + (R * np.arange(NCORES, dtype=np.int64))[:, None, None]) % N

    v = vals.reshape(N, 8 * NQ)
    jg = j_glob.reshape(N, 8 * NQ)
    order = np.lexsort((jg, -v.astype(np.float64)), axis=-1)
    top8_j = np.take_along_axis(jg, order, axis=-1)[:, :8]
    top8_v = np.take_along_axis(v, order, axis=-1)[:, :8]

    srt = np.sort(top8_j, axis=1)
    bad = (srt[:, 1:] == srt[:, :-1]).any(axis=1)
    bad |= (top8_v <= NEG_BIG / 2).any(axis=1)
    bad |= (top8_j == np.arange(N)[:, None]).any(axis=1)
    if bad.any():
        fix = _recompute_rows_f64(x, np.where(bad)[0])
        for r, idx in fix.items():
            top8_j[r] = idx

    nn_idx = np.empty((N, 9), dtype=np.int32)
    nn_idx[:, 0] = np.arange(N, dtype=np.int32)
    nn_idx[:, 1:] = top8_j.astype(np.int32)
    return nn_idx


def kernel(x, k):
    x = np.ascontiguousarray(np.asarray(x, dtype=np.float32))
    k = int(np.asarray(k))
    assert x.shape == (N, D) and k == 9, (x.shape, k)

    nc = _get_nc()
    in_maps = []
    for c in range(NCORES):
        if c == 0:
            xr = x
        else:
            xr = np.ascontiguousarray(
                np.concatenate([x[R * c:], x[:R * c]], axis=0))
        in_maps.append({"xin": xr})

    t0 = time.time()
    res = run_bass_kernel_spmd(nc, in_maps, core_ids=list(range(NCORES)))
    _CACHE["last_res"] = res
    _CACHE["last_exec_wall_s"] = time.time() - t0
    vals = np.stack([r["vals_out"] for r in res.results])   # [8, 1250, 32]
    idxs = np.stack([r["idx_out"] for r in res.results])    # [8, 1250, 32]

    nn_idx = _merge(vals, idxs, x)
    center = np.broadcast_to(np.arange(N, dtype=np.int32)[:, None], (N, 9))
    return np.stack([nn_idx, np.ascontiguousarray(center)], axis=0)


if __name__ == "__main__":
    build_nc()
    print("built OK")


# revision 13
# speedup vs baseline: 1.0051x; 1.0051x over previous
"""Dense dilated KNN graph kernel for Trainium2 (8 NeuronCores).

Problem: x [10000, 512] f32, k=9.
reference: column-L2-normalize x (dim=0) -> xn; dist_ij = ||xn_i - xn_j||^2;
edge_idx = stack((top9_idx(-dist), center_idx)) -> [2, 10000, 9] int32.

Math: row i's k-NN ranking of -dist equals the DESCENDING ranking of
score(i,j) = xn_i . xn_j - ||xn_j||^2/2  (the sq_i term is constant per row).
score(i,i) is always the row max (score_ii - score_ij = ||xn_i-xn_j||^2/2 >= 0),
so top-9 = [self] + top-8 of the rest, and k=9 maps exactly onto the DVE's
8-wide max/max_index ops after knocking out the diagonal.

Precision: fp32 matmuls cannot PSUM-accumulate on this toolchain, and a plain
bf16 matmul is far too coarse for the ~1e-5 gaps between neighbor distances.
So xn is stored as a bf16 hi/lo pair (xn ~ hi + lo, |lo| <~ 2^-9 |xn|) and
G = hi@hi.T + hi@lo.T + lo@hi.T, giving ~3e-8 absolute score error (fp32
class) at full bf16 matmul speed.

Sharding: rows split across 8 cores (1250 each). Every core receives a
ROTATED copy of x (rolled by -1250*c) so one SPMD program serves all cores:
"my" rows are local rows 0..1249 and the self-diagonal lands at
compile-time-known positions.

Per core:
  pass 1: load x, PE-transpose, ACT Square-accumulate -> column norms
  pass 2: reload x, normalize (DVE), split to bf16 hi/lo, PE-transpose into
          hi_c/lo_c [128, 10000] x 4 chunks; ACT Square-accum gives row
          norms sq_j for free; sq encoded as bf16 hi+lo rows [2, 10000]
  main:   per (row-tile 125 x col-chunk 500): 12 bf16 matmuls + sq aug-row
          matmul + diagonal-knockout matmul accumulate score into PSUM;
          ACT evacuates into score quarters [125, 2500]; DVE max/max_index
          produce top-8 per quarter -> 32 candidates/row to DRAM.
Host: merge candidates, take top-8 by (value desc, index asc), prepend self.
"""

import time

import numpy as np
import ml_dtypes

import concourse.bacc as bacc
import concourse.mybir as mybir
import concourse.tile as tile
from concourse.masks import make_identity
from concourse.bass_utils import run_bass_kernel_spmd

N = 10000
D = 512
NCORES = 8
R = N // NCORES          # 1250 rows per core
TM = 125                 # row-tile size (PSUM out partitions)
NT = R // TM             # 10 row tiles
W = 500                  # col chunk (one PSUM bank at fp32)
NCH = N // W             # 20 col chunks
QW = 2500                # score quarter width
NQ = N // QW             # 4 quarters
CPQ = QW // W            # 5 chunks per quarter
KC = D // 128            # 4 contraction chunks
G = (N + 127) // 128     # 79 row groups for load/transpose (78 full + 16)

F32 = mybir.dt.float32
BF16 = mybir.dt.bfloat16
U32 = mybir.dt.uint32
COPY = mybir.ActivationFunctionType.Copy
SQUARE = mybir.ActivationFunctionType.Square
SQRT = mybir.ActivationFunctionType.Sqrt

NEG_BIG = -1e30

_CACHE = {}


def build_nc():
    nc = bacc.Bacc("TRN2", target_bir_lowering=False, debug=False,
                   num_devices=NCORES)

    xin = nc.dram_tensor("xin", [N, D], F32, kind="ExternalInput")
    vals_out = nc.dram_tensor("vals_out", [R, 8 * NQ], F32, kind="ExternalOutput")
    idx_out = nc.dram_tensor("idx_out", [R, 8 * NQ], U32, kind="ExternalOutput")
    # DRAM scratch for layout shuffles (partition-dim <-> free-dim folds)
    dinv = nc.dram_tensor("dinv", [KC, 128], F32)
    dsq = nc.dram_tensor("dsq", [2, G * 128], BF16)

    with tile.TileContext(nc) as tc:
        with (
            tc.tile_pool(name="big", bufs=1) as big,
            tc.tile_pool(name="xt", bufs=2) as xtp,
            tc.tile_pool(name="f32s", bufs=2) as f32s,
            tc.tile_pool(name="bf16s", bufs=2) as bf16s,
            tc.tile_pool(name="outs", bufs=4) as outp,
            tc.tile_pool(name="pt", bufs=2, space="PSUM") as ptp,
            tc.tile_pool(name="prep", bufs=1, space="PSUM") as prep,
            tc.tile_pool(name="pm", bufs=5, space="PSUM") as pmp,
        ):
            # ---- constants ----
            identf = big.tile([128, 128], F32, tag="identf")
            make_identity(nc, identf)
            identb = big.tile([128, 128], BF16, tag="identb")
            nc.vector.tensor_copy(identb, identf)
            negeye = big.tile([128, 128], BF16, tag="negeye")
            nc.vector.tensor_scalar_mul(negeye, identb, NEG_BIG)
            ones2 = big.tile([2, TM], BF16, tag="ones2")
            nc.vector.memset(ones2, 1.0)
            ones_k1 = big.tile([1, 128], F32, tag="ones_k1")
            nc.vector.memset(ones_k1, 1.0)

            # ---- persistent big buffers ----
            hi = [big.tile([128, N], BF16, tag=f"hi{c}", name=f"hi{c}")
                  for c in range(KC)]
            lo = [big.tile([128, N], BF16, tag=f"lo{c}", name=f"lo{c}")
                  for c in range(KC)]
            sqh = big.tile([2, N], BF16, tag="sqh")          # -sq/2 hi, lo rows
            score = big.tile([128, QW], F32, tag="score", name="score")
            part = [big.tile([128, G], F32, tag=f"part{c}", name=f"part{c}")
                    for c in range(KC)]                      # colnorm partials
            cn = big.tile([128, KC], F32, tag="cn")
            inv = big.tile([128, KC], F32, tag="inv")
            invT = big.tile([KC, 128], F32, tag="invT")
            invrep = big.tile([128, D], F32, tag="invrep")
            sq_nat = big.tile([128, G], F32, tag="sq_nat")   # row norms, [p, g]
            nc.vector.memset(sq_nat, 0.0)  # tail of last group never written
            sq79 = big.tile([128, G], F32, tag="sq79")       # -sq/2 f32
            hi79 = big.tile([128, G], BF16, tag="hi79")
            lo79 = big.tile([128, G], BF16, tag="lo79")
            hi79f = big.tile([128, G], F32, tag="hi79f")
            sqT = big.tile([G, 128], BF16, tag="sqT")
            sqT2 = big.tile([G, 128], BF16, tag="sqT2")

            # ---- pass 1: column norms ----
            # transpose raw x blocks; ACT Square-accum along free dim (rows)
            for g in range(G):
                r0 = 128 * g
                rn = min(128, N - r0)
                xt = xtp.tile([128, D], F32, tag="xt")
                nc.sync.dma_start(xt[:rn, :], xin[r0:r0 + rn, :])
                for c in range(KC):
                    pt = ptp.tile([128, 128], F32, tag="pt")
                    nc.tensor.transpose(pt[:, :rn], xt[:rn, 128 * c:128 * (c + 1)],
                                        identf[:rn, :rn])
                    dump = f32s.tile([128, D], F32, tag="xn32", name="dump")
                    nc.scalar.activation(dump[:, :rn], pt[:, :rn], SQUARE,
                                         accum_out=part[c][:, g:g + 1])

            # finalize column norms -> inv = 1/max(sqrt(sum), eps)
            for c in range(KC):
                nc.vector.tensor_reduce(cn[:, c:c + 1], part[c],
                                        axis=mybir.AxisListType.X,
                                        op=mybir.AluOpType.add)
            nc.scalar.activation(cn, cn, SQRT)
            nc.vector.tensor_scalar_max(cn, cn, 1e-12)
            nc.vector.reciprocal(inv, cn)

            # replicate inv over partitions in natural layout:
            # inv [128,4] -T-> invT [4,128] -DRAM-> flat row -> K=1 matmul bcast
            pti = ptp.tile([KC, 128], F32, tag="pt")
            nc.tensor.transpose(pti, inv, identf)
            nc.scalar.activation(invT, pti, COPY)
            nc.sync.dma_start(dinv[:], invT)
            invrow = score[0:1, 0:D]         # score buf is free in the prologue
            nc.sync.dma_start(invrow, dinv.ap().rearrange("a b -> (a b)")[None, :])
            pri = prep.tile([128, D], F32, tag="prep")
            nc.tensor.matmul(pri, ones_k1, invrow, start=True, stop=True)
            nc.scalar.activation(invrep, pri, COPY)

            # ---- pass 2: normalize, split hi/lo, transpose; row norms ----
            for g in range(G):
                r0 = 128 * g
                rn = min(128, N - r0)
                xt = xtp.tile([128, D], F32, tag="xt")
                nc.sync.dma_start(xt[:rn, :], xin[r0:r0 + rn, :])
                xn32 = f32s.tile([128, D], F32, tag="xn32")
                nc.vector.tensor_mul(xn32[:rn, :], xt[:rn, :], invrep[:rn, :])
                hin = bf16s.tile([128, D], BF16, tag="hin")
                nc.vector.tensor_copy(hin[:rn, :], xn32[:rn, :])
                # raw x in xt is dead after the mul above -- reuse as f32 scratch
                nc.vector.tensor_copy(xt[:rn, :], hin[:rn, :])
                lon = bf16s.tile([128, D], BF16, tag="lon")
                nc.vector.tensor_sub(lon[:rn, :], xn32[:rn, :], xt[:rn, :])
                # row-norm contribution (free-dim accum over all 512 d);
                # xt is dead again -- reuse it as the ACT dump
                nc.scalar.activation(xt[:rn, :], xn32[:rn, :], SQUARE,
                                     accum_out=sq_nat[:rn, g:g + 1])
                for c in range(KC):
                    cs = slice(128 * c, 128 * (c + 1))
                    pt = ptp.tile([128, 128], BF16, tag="pt", name="ptb")
                    nc.tensor.transpose(pt[:, :rn], hin[:rn, cs], identb[:rn, :rn])
                    nc.scalar.activation(hi[c][:, r0:r0 + rn], pt[:, :rn], COPY)
                    pt2 = ptp.tile([128, 128], BF16, tag="pt", name="ptb2")
                    nc.tensor.transpose(pt2[:, :rn], lon[:rn, cs], identb[:rn, :rn])
                    nc.scalar.activation(lo[c][:, r0:r0 + rn], pt2[:, :rn], COPY)

            # ---- sq rows: -sq/2 as bf16 hi+lo in [2, N] layout ----
            nc.vector.tensor_scalar_mul(sq79, sq_nat, -0.5)
            nc.vector.tensor_scalar_mul(hi79, sq_nat, -0.5)   # cast to bf16
            nc.vector.tensor_copy(hi79f, hi79)
            nc.vector.tensor_sub(lo79, sq79, hi79f)
            ptq = ptp.tile([G, 128], BF16, tag="pt", name="ptq")
            nc.tensor.transpose(ptq, hi79, identb)
            nc.scalar.activation(sqT, ptq, COPY)
            ptq2 = ptp.tile([G, 128], BF16, tag="pt", name="ptq2")
            nc.tensor.transpose(ptq2, lo79, identb)
            nc.scalar.activation(sqT2, ptq2, COPY)
            nc.sync.dma_start(dsq[0:1, :].rearrange("a (g r) -> (a g) r", g=G), sqT)
            nc.sync.dma_start(dsq[1:2, :].rearrange("a (g r) -> (a g) r", g=G), sqT2)
            nc.sync.dma_start(sqh[0:1, :], dsq[0:1, 0:N])
            nc.sync.dma_start(sqh[1:2, :], dsq[1:2, 0:N])

            # ---- main loop ----
            for t in range(NT):
                rbase = TM * t
                diag_ch = rbase // W
                diag_off = rbase - W * diag_ch
                for q in range(NQ):
                    sc = score
                    for j in range(CPQ):
                        n = CPQ * q + j
                        ns = slice(W * n, W * (n + 1))
                        rs = slice(rbase, rbase + TM)
                        pm = pmp.tile([TM, W], F32, tag="pm")
                        for c in range(KC):
                            nc.tensor.matmul(pm, hi[c][:, rs], hi[c][:, ns],
                                             start=(c == 0), stop=False)
                            nc.tensor.matmul(pm, hi[c][:, rs], lo[c][:, ns],
                                             start=False, stop=False)
                            nc.tensor.matmul(pm, lo[c][:, rs], hi[c][:, ns],
                                             start=False, stop=False)
                        is_diag = (n == diag_ch)
                        nc.tensor.matmul(pm, ones2, sqh[:, ns],
                                         start=False, stop=not is_diag)
                        if is_diag:
                            nc.tensor.matmul(
                                pm[:, diag_off:diag_off + TM],
                                identb[:TM, :TM], negeye[:TM, :TM],
                                start=False, stop=True)
                        nc.scalar.activation(sc[:TM, W * j:W * (j + 1)], pm, COPY)
                    mval = outp.tile([TM, 8], F32, tag="mv")
                    midx = outp.tile([TM, 8], U32, tag="mi")
                    nc.vector.max(out=mval, in_=sc[:TM, :])
                    nc.vector.max_index(out=midx, in_max=mval, in_values=sc[:TM, :])
                    nc.sync.dma_start(vals_out[rs, 8 * q:8 * (q + 1)], mval)
                    nc.sync.dma_start(idx_out[rs, 8 * q:8 * (q + 1)], midx)

    nc.compile()
    return nc


def _get_nc():
    if "nc" not in _CACHE:
        _CACHE["nc"] = build_nc()
    return _CACHE["nc"]


def _recompute_rows_f64(x, rows):
    """Exact f64 top-8 (excluding self) for the given rows."""
    xf = x.astype(np.float64)
    cnorm = np.sqrt((xf * xf).sum(0, keepdims=True))
    xn = xf / np.maximum(cnorm, 1e-12)
    sq = (xn * xn).sum(1)
    out = {}
    for r in rows:
        s = xn[r] @ xn.T - sq / 2.0
        s[r] = -np.inf
        idx = np.lexsort((np.arange(N), -s))[:8]
        out[r] = idx
    return out


GAP_TAU = 2e-6   # rescore rows whose candidate ranking is this close


def _merge(vals, idxs, x):
    """vals/idxs: [NCORES, R, 32] device candidates -> nn_idx [N, 9] int32."""
    q_off = (np.arange(8 * NQ, dtype=np.int64) // 8) * QW
    j_loc = idxs.astype(np.int64) + q_off[None, None, :]
    j_glob = (j_loc + (R * np.arange(NCORES, dtype=np.int64))[:, None, None]) % N

    v = vals.reshape(N, 8 * NQ)
    jg = j_glob.reshape(N, 8 * NQ)
    order = np.lexsort((jg, -v.astype(np.float64)), axis=-1)
    sv = np.take_along_axis(v, order, axis=-1).astype(np.float64)
    sj = np.take_along_axis(jg, order, axis=-1)
    top8_j = sj[:, :8].copy()
    top8_v = sv[:, :8]

    # rows whose top-9 candidate scores contain a near-tie: the device's
    # ~1e-7 score noise can order them differently from the reference, so
    # re-rank those rows' candidates with exact f64 scores.
    close = (sv[:, :8] - sv[:, 1:9]) < GAP_TAU
    bad = close.any(axis=1)
    # defensive: duplicate winners / knockout leakage / self leakage
    srt = np.sort(top8_j, axis=1)
    hard_bad = (srt[:, 1:] == srt[:, :-1]).any(axis=1)
    hard_bad |= (top8_v <= NEG_BIG / 2).any(axis=1)
    hard_bad |= (top8_j == np.arange(N)[:, None]).any(axis=1)

    if bad.any():
        rows = np.where(bad & ~hard_bad)[0]
        if rows.size:
            xf = x.astype(np.float64)
            cnorm = np.sqrt((xf * xf).sum(0, keepdims=True))
            xn = xf / np.maximum(cnorm, 1e-12)
            sq = (xn * xn).sum(1)
            cand = sj[rows]                       # [nr, 32]
            s64 = np.einsum("rd,rcd->rc", xn[rows], xn[cand]) - sq[cand] / 2.0
            oo = np.lexsort((cand, -s64), axis=-1)
            top8_j[rows] = np.take_along_axis(cand, oo, axis=-1)[:, :8]
    if hard_bad.any():
        fix = _recompute_rows_f64(x, np.where(hard_bad)[0])
        for r, idx in fix.items():
            top8_j[r] = idx

    nn_idx = np.empty((N, 9), dtype=np.int32)
    nn_idx[:, 0] = np.arange(N, dtype=np.int32)
    nn_idx[:, 1:] = top8_j.astype(np.int32)
    return nn_idx


def kernel(x, k):
    x = np.ascontiguousarray(np.asarray(x, dtype=np.float32))
    k = int(np.asarray(k))
    assert x.shape == (N, D) and k == 9, (x.shape, k)

    nc = _get_nc()
    in_maps = []
    for c in range(NCORES):
        if c == 0:
            xr = x
        else:
            xr = np.ascontiguousarray(
                np.concatenate([x[R * c:], x[:R * c]], axis=0))
        in_maps.append({"xin": xr})

    t0 = time.time()
    res = run_bass_kernel_spmd(nc, in_maps, core_ids=list(range(NCORES)))
    _CACHE["last_res"] = res
    _CACHE["last_exec_wall_s"] = time.time() - t0
    vals = np.stack([r["vals_out"] for r in res.results])   # [8, 1250, 32]
    idxs = np.stack([r["idx_out"] for r in res.results])    # [8, 1250, 32]

    nn_idx = _merge(vals, idxs, x)
    center = np.broadcast_to(np.arange(N, dtype=np.int32)[:, None], (N, 9))
    return np.stack([nn_idx, np.ascontiguousarray(center)], axis=0)


if __name__ == "__main__":
    build_nc()
    print("built OK")


# revision 25
# speedup vs baseline: 1.0349x; 1.0297x over previous
"""Dense dilated KNN graph kernel for Trainium2 (8 NeuronCores).

Problem: x [10000, 512] f32, k=9.
reference: column-L2-normalize x (dim=0) -> xn; dist_ij = ||xn_i - xn_j||^2;
edge_idx = stack((top9_idx(-dist), center_idx)) -> [2, 10000, 9] int32.

Math: row i's k-NN ranking of -dist equals the DESCENDING ranking of
score(i,j) = xn_i . xn_j - ||xn_j||^2/2  (the sq_i term is constant per row).
score(i,i) is always the row max (score_ii - score_ij = ||xn_i-xn_j||^2/2 >= 0),
so top-9 = [self] + top-8 of the rest, and k=9 maps exactly onto the DVE's
8-wide max/max_index ops after knocking out the diagonal.

Precision: fp32 matmuls cannot PSUM-accumulate on this toolchain, and a plain
bf16 matmul is far too coarse for the ~1e-5 gaps between neighbor distances.
So xn is stored as a bf16 hi/lo pair (xn ~ hi + lo, |lo| <~ 2^-9 |xn|) and
G = hi@hi.T + hi@lo.T + lo@hi.T, giving ~3e-8 absolute score error (fp32
class) at full bf16 matmul speed.

Sharding: rows split across 8 cores (1250 each). Every core receives a
ROTATED copy of x (rolled by -1250*c) so one SPMD program serves all cores:
"my" rows are local rows 0..1249 and the self-diagonal lands at
compile-time-known positions.

Per core:
  pass 1: load x, PE-transpose, ACT Square-accumulate -> column norms
  pass 2: reload x, normalize (DVE), split to bf16 hi/lo, PE-transpose into
          hi_c/lo_c [128, 10000] x 4 chunks; ACT Square-accum gives row
          norms sq_j for free; sq encoded as bf16 hi+lo rows [2, 10000]
  main:   per (row-tile 125 x col-chunk 500): 12 bf16 matmuls + sq aug-row
          matmul + diagonal-knockout matmul accumulate score into PSUM;
          ACT evacuates into score quarters [125, 2500]; DVE max/max_index
          produce top-8 per quarter -> 32 candidates/row to DRAM.
Host: merge candidates, take top-8 by (value desc, index asc), prepend self.
"""

import time

import numpy as np
import ml_dtypes

import concourse.bacc as bacc
import concourse.mybir as mybir
import concourse.tile as tile
from concourse.masks import make_identity
from concourse.bass_utils import run_bass_kernel_spmd

N = 10000
D = 512
NCORES = 8
R = N // NCORES          # 1250 rows per core
TM = 125                 # row-tile size (PSUM out partitions)
NT = R // TM             # 10 row tiles
W = 500                  # col chunk (one PSUM bank at fp32)
NCH = N // W             # 20 col chunks
HN = N // 2              # column half width (overlap split)
QW = 2500                # score quarter width
NQ = N // QW             # 4 quarters
CPQ = QW // W            # 5 chunks per quarter
KC = D // 128            # 4 contraction chunks
G = (N + 127) // 128     # 79 row groups for load/transpose (78 full + 16)
GB = 8                   # row-groups batched per PSUM tile in the prologue
NB = (G + GB - 1) // GB  # 20 batches

F32 = mybir.dt.float32
BF16 = mybir.dt.bfloat16
U32 = mybir.dt.uint32
COPY = mybir.ActivationFunctionType.Copy
SQUARE = mybir.ActivationFunctionType.Square
SQRT = mybir.ActivationFunctionType.Sqrt

NEG_BIG = -1e30

_CACHE = {}


def build_nc():
    nc = bacc.Bacc("TRN2", target_bir_lowering=False, debug=False,
                   num_devices=NCORES)

    xin = nc.dram_tensor("xin", [N, D], F32, kind="ExternalInput")
    vals_out = nc.dram_tensor("vals_out", [R, 8 * NQ], F32, kind="ExternalOutput")
    idx_out = nc.dram_tensor("idx_out", [R, 8 * NQ], U32, kind="ExternalOutput")
    # DRAM scratch for layout shuffles (partition-dim <-> free-dim folds)
    dinv = nc.dram_tensor("dinv", [KC, 128], F32)
    GA = (HN + 127) // 128          # 40 groups feed half A's sq rows
    GBH = G - GA                    # 39 groups in half B
    dsq = [nc.dram_tensor(f"dsq{h}", [2, (GA, GBH)[h] * 128], BF16)
           for h in range(2)]

    with tile.TileContext(nc) as tc:
        with (
            tc.tile_pool(name="big", bufs=1) as big,
            tc.tile_pool(name="xt", bufs=8) as xtp,
            tc.tile_pool(name="outs", bufs=2) as outp,
            tc.tile_pool(name="pt", bufs=2, space="PSUM") as ptp,
            tc.tile_pool(name="pm", bufs=4, space="PSUM") as pmp,
        ):
            # ---- constants ----
            identf = big.tile([128, 128], F32, tag="identf")
            make_identity(nc, identf)
            identb = big.tile([128, 128], BF16, tag="identb")
            nc.vector.tensor_copy(identb, identf)
            negeye = big.tile([128, 128], BF16, tag="negeye")
            nc.vector.tensor_scalar_mul(negeye, identb, NEG_BIG)
            ones2 = big.tile([2, TM], BF16, tag="ones2")
            nc.vector.memset(ones2, 1.0)


            # ---- persistent big buffers (column-halved for overlap) ----
            hi = {(c, h): big.tile([128, HN], BF16, tag=f"hi{c}_{h}",
                                   name=f"hi{c}_{h}")
                  for c in range(KC) for h in range(2)}
            lo = {(c, h): big.tile([128, HN], BF16, tag=f"lo{c}_{h}",
                                   name=f"lo{c}_{h}")
                  for c in range(KC) for h in range(2)}
            sqh = [big.tile([2, HN], BF16, tag=f"sqh{h}", name=f"sqh{h}")
                   for h in range(2)]
            score = big.tile([128, QW], F32, tag="score", name="score")
            part = [big.tile([128, NB], F32, tag=f"part{c}", name=f"part{c}")
                    for c in range(KC)]
            cn = big.tile([128, KC], F32, tag="cn")
            inv = big.tile([128, KC], F32, tag="inv")
            invrep = big.tile([128, D], F32, tag="invrep")
            sq_nat = [big.tile([128, (GA, GBH)[h]], F32, tag=f"sq_nat{h}",
                               name=f"sq_nat{h}") for h in range(2)]
            nc.vector.memset(sq_nat[1], 0.0)   # tail of last group never written
            sq79 = [big.tile([128, (GA, GBH)[h]], F32, tag=f"sq79{h}",
                             name=f"sq79{h}") for h in range(2)]
            hi79 = [big.tile([128, (GA, GBH)[h]], BF16, tag=f"hi79{h}",
                             name=f"hi79{h}") for h in range(2)]
            lo79 = [big.tile([128, (GA, GBH)[h]], BF16, tag=f"lo79{h}",
                             name=f"lo79{h}") for h in range(2)]
            sqT = [big.tile([(GA, GBH)[h], 128], BF16, tag=f"sqT{h}",
                            name=f"sqT{h}") for h in range(2)]
            sqT2 = [big.tile([(GA, GBH)[h], 128], BF16, tag=f"sqT2{h}",
                             name=f"sqT2{h}") for h in range(2)]

            def load_eng(i):
                return nc.sync if i % 2 == 0 else nc.scalar

            # ---- pass 1: column norms ----
            # transpose raw x blocks (8 row-groups per 2-bank PSUM tile);
            # square-reduce along rows on DVE (c<2) / ACT (c>=2), in place
            for b in range(NB):
                gs = list(range(GB * b, min(GB * b + GB, G)))
                xts = []
                for i, g in enumerate(gs):
                    r0 = 128 * g
                    rn = min(128, N - r0)
                    xt = xtp.tile([128, D], F32, tag="xt", name="xt")
                    load_eng(i).dma_start(xt[:rn, :], xin[r0:r0 + rn, :])
                    xts.append((xt, rn))
                used = sum(rn for _, rn in xts)
                for c in range(KC):
                    cs = slice(128 * c, 128 * (c + 1))
                    pt = ptp.tile([128, GB * 128], F32, tag="pt", name="pt1")
                    off = 0
                    for xt, rn in xts:
                        nc.tensor.transpose(pt[:, off:off + rn], xt[:rn, cs],
                                            identf[:rn, :rn])
                        off += rn
                    # squares overwrite the transposed block in place; pt is
                    # dead after (single-input ACT op: the DVE cannot read
                    # two PSUM operands)
                    nc.scalar.activation(pt[:, :used], pt[:, :used], SQUARE,
                                         accum_out=part[c][:, b:b + 1])

            # finalize column norms -> inv = 1/max(sqrt(sum), eps)
            for c in range(KC):
                nc.vector.tensor_reduce(cn[:, c:c + 1], part[c],
                                        axis=mybir.AxisListType.X,
                                        op=mybir.AluOpType.add)
            nc.scalar.activation(cn, cn, SQRT)
            nc.vector.tensor_scalar_max(cn, cn, 1e-12)
            nc.vector.reciprocal(inv, cn)

            # replicate inv over partitions in natural layout:
            # inv [128,4] -T-> invT [4,128] -DRAM-> flat row -> K=1 matmul bcast
            # (the score buffer is free real estate during the prologue)
            invT = score[0:KC, 0:128]
            ones_k1 = score[0:1, 2 * D:2 * D + 128]
            nc.vector.memset(ones_k1, 1.0)
            pti = ptp.tile([KC, 128], F32, tag="pt", name="pti")
            nc.tensor.transpose(pti, inv, identf)
            nc.scalar.activation(invT, pti, COPY)
            nc.sync.dma_start(dinv[:], invT)
            invrow = score[0:1, D:2 * D]
            nc.sync.dma_start(invrow, dinv.ap().rearrange("a b -> (a b)")[None, :])
            pri = ptp.tile([128, D], F32, tag="pt", name="pri")
            nc.tensor.matmul(pri, ones_k1, invrow, start=True, stop=True)
            nc.scalar.activation(invrep, pri, COPY)

            # ---- pass 2 (per half): normalize, transpose, split hi/lo ----
            def pass2_batch(b):
                gs = list(range(GB * b, min(GB * b + GB, G)))
                c0 = 128 * GB * b              # first column this batch writes
                dump = ptp.tile([128, GB * 128], F32, tag="pt", name="ptd")
                xts = []
                for i, g in enumerate(gs):
                    r0 = 128 * g
                    rn = min(128, N - r0)
                    xt = xtp.tile([128, D], F32, tag="xt", name="xt")
                    load_eng(i).dma_start(xt[:rn, :], xin[r0:r0 + rn, :])
                    # normalize in place on the (otherwise idle) GPSIMD
                    nc.gpsimd.tensor_mul(xt[:rn, :], xt[:rn, :], invrep[:rn, :])
                    h, gh = (0, g) if g < GA else (1, g - GA)
                    nc.scalar.activation(dump[:rn, (i % 2) * D:(i % 2 + 1) * D],
                                         xt[:rn, :], SQUARE,
                                         accum_out=sq_nat[h][:rn, gh:gh + 1])
                    xts.append((xt, rn))
                used = sum(rn for _, rn in xts)
                # split of this batch's columns between the two halves
                parts = []
                if c0 < HN:
                    parts.append((0, 0, min(used, HN - c0), c0))
                if c0 + used > HN:
                    s = max(0, HN - c0)
                    parts.append((1, s, used, c0 + s - HN))
                for c in range(KC):
                    cs = slice(128 * c, 128 * (c + 1))
                    pt = ptp.tile([128, GB * 128], F32, tag="pt", name="pt2")
                    off = 0
                    for xt, rn in xts:
                        nc.tensor.transpose(pt[:, off:off + rn], xt[:rn, cs],
                                            identf[:rn, :rn])
                        off += rn
                    for h, a, bnd, dst in parts:
                        w = bnd - a
                        nc.scalar.activation(hi[(c, h)][:, dst:dst + w],
                                             pt[:, a:bnd], COPY)
                        nc.vector.tensor_sub(lo[(c, h)][:, dst:dst + w],
                                             pt[:, a:bnd],
                                             hi[(c, h)][:, dst:dst + w])

            def sq_finalize(h):
                gh = (GA, GBH)[h]
                nc.vector.tensor_scalar_mul(sq79[h], sq_nat[h], -0.5)
                nc.vector.tensor_scalar_mul(hi79[h], sq_nat[h], -0.5)  # ->bf16
                nc.vector.tensor_sub(lo79[h], sq79[h], hi79[h])
                ptq = ptp.tile([gh, 128], BF16, tag="pt", name=f"ptq{h}")
                nc.tensor.transpose(ptq, hi79[h], identb)
                nc.scalar.activation(sqT[h], ptq, COPY)
                ptq2 = ptp.tile([gh, 128], BF16, tag="pt", name=f"ptq2{h}")
                nc.tensor.transpose(ptq2, lo79[h], identb)
                nc.scalar.activation(sqT2[h], ptq2, COPY)
                dq = dsq[h]
                nc.sync.dma_start(
                    dq[0:1, :].rearrange("a (g r) -> (a g) r", g=gh), sqT[h])
                nc.sync.dma_start(
                    dq[1:2, :].rearrange("a (g r) -> (a g) r", g=gh), sqT2[h])
                if h == 0:
                    for row in range(2):
                        nc.sync.dma_start(sqh[0][row:row + 1, :],
                                          dsq[0][row:row + 1, 0:HN])
                else:
                    # rows 5000..5120 come from half A's tail group
                    for row in range(2):
                        nc.sync.dma_start(sqh[1][row:row + 1, 0:GA * 128 - HN],
                                          dsq[0][row:row + 1, HN:GA * 128])
                        nc.sync.dma_start(sqh[1][row:row + 1, GA * 128 - HN:HN],
                                          dsq[1][row:row + 1, 0:N - GA * 128])

            def main_phase(ph):
                for t in range(NT):
                    rbase = TM * t
                    diag_ch = rbase // W
                    diag_off = rbase - W * diag_ch
                    for q in (2 * ph, 2 * ph + 1):
                        h = (QW * q) // HN
                        colbase = HN * h
                        for j in range(CPQ):
                            n = CPQ * q + j
                            ns = slice(W * n - colbase, W * (n + 1) - colbase)
                            rs = slice(rbase, rbase + TM)
                            pm = pmp.tile([TM, W], F32, tag="pm")
                            for c in range(KC):
                                nc.tensor.matmul(pm, hi[(c, 0)][:, rs],
                                                 hi[(c, h)][:, ns],
                                                 start=(c == 0), stop=False)
                                nc.tensor.matmul(pm, hi[(c, 0)][:, rs],
                                                 lo[(c, h)][:, ns],
                                                 start=False, stop=False)
                                nc.tensor.matmul(pm, lo[(c, 0)][:, rs],
                                                 hi[(c, h)][:, ns],
                                                 start=False, stop=False)
                            is_diag = (n == diag_ch)   # always in half 0
                            nc.tensor.matmul(pm, ones2, sqh[h][:, ns],
                                             start=False, stop=not is_diag)
                            if is_diag:
                                nc.tensor.matmul(
                                    pm[:, diag_off:diag_off + TM],
                                    identb[:TM, :TM], negeye[:TM, :TM],
                                    start=False, stop=True)
                            nc.scalar.activation(score[:TM, W * j:W * (j + 1)],
                                                 pm, COPY)
                        mval = outp.tile([TM, 8], F32, tag="mv")
                        midx = outp.tile([TM, 8], U32, tag="mi")
                        nc.vector.max(out=mval, in_=score[:TM, :])
                        nc.vector.max_index(out=midx, in_max=mval,
                                            in_values=score[:TM, :])
                        rsl = slice(rbase, rbase + TM)
                        nc.sync.dma_start(vals_out[rsl, 8 * q:8 * (q + 1)], mval)
                        nc.sync.dma_start(idx_out[rsl, 8 * q:8 * (q + 1)], midx)

            NBA = (GA + GB - 1) // GB          # batches fully needed by half A
            for b in range(NBA):
                pass2_batch(b)
            sq_finalize(0)
            for b in range(NBA, NB):
                pass2_batch(b)
            sq_finalize(1)
            main_phase(0)
            main_phase(1)

    nc.compile()
    return nc


def _get_nc():
    if "nc" not in _CACHE:
        _CACHE["nc"] = build_nc()
    return _CACHE["nc"]


def _recompute_rows_f64(x, rows):
    """Exact f64 top-8 (excluding self) for the given rows."""
    xf = x.astype(np.float64)
    cnorm = np.sqrt((xf * xf).sum(0, keepdims=True))
    xn = xf / np.maximum(cnorm, 1e-12)
    sq = (xn * xn).sum(1)
    out = {}
    for r in rows:
        s = xn[r] @ xn.T - sq / 2.0
        s[r] = -np.inf
        idx = np.lexsort((np.arange(N), -s))[:8]
        out[r] = idx
    return out


GAP_TAU = 2e-6   # rescore rows whose candidate ranking is this close


def _merge(vals, idxs, x):
    """vals/idxs: [NCORES, R, 32] device candidates -> nn_idx [N, 9] int32."""
    q_off = (np.arange(8 * NQ, dtype=np.int64) // 8) * QW
    j_loc = idxs.astype(np.int64) + q_off[None, None, :]
    j_glob = (j_loc + (R * np.arange(NCORES, dtype=np.int64))[:, None, None]) % N

    v = vals.reshape(N, 8 * NQ)
    jg = j_glob.reshape(N, 8 * NQ)
    order = np.lexsort((jg, -v.astype(np.float64)), axis=-1)
    sv = np.take_along_axis(v, order, axis=-1).astype(np.float64)
    sj = np.take_along_axis(jg, order, axis=-1)
    top8_j = sj[:, :8].copy()
    top8_v = sv[:, :8]

    # rows whose top-9 candidate scores contain a near-tie: the device's
    # ~1e-7 score noise can order them differently from the reference, so
    # re-rank those rows' candidates with exact f64 scores.
    close = (sv[:, :8] - sv[:, 1:9]) < GAP_TAU
    bad = close.any(axis=1)
    # defensive: duplicate winners / knockout leakage / self leakage
    srt = np.sort(top8_j, axis=1)
    hard_bad = (srt[:, 1:] == srt[:, :-1]).any(axis=1)
    hard_bad |= (top8_v <= NEG_BIG / 2).any(axis=1)
    hard_bad |= (top8_j == np.arange(N)[:, None]).any(axis=1)

    if bad.any():
        rows = np.where(bad & ~hard_bad)[0]
        if rows.size:
            xf = x.astype(np.float64)
            cnorm = np.sqrt((xf * xf).sum(0, keepdims=True))
            xn = xf / np.maximum(cnorm, 1e-12)
            sq = (xn * xn).sum(1)
            cand = sj[rows]                       # [nr, 32]
            s64 = np.einsum("rd,rcd->rc", xn[rows], xn[cand]) - sq[cand] / 2.0
            oo = np.lexsort((cand, -s64), axis=-1)
            top8_j[rows] = np.take_along_axis(cand, oo, axis=-1)[:, :8]
    if hard_bad.any():
        fix = _recompute_rows_f64(x, np.where(hard_bad)[0])
        for r, idx in fix.items():
            top8_j[r] = idx

    nn_idx = np.empty((N, 9), dtype=np.int32)
    nn_idx[:, 0] = np.arange(N, dtype=np.int32)
    nn_idx[:, 1:] = top8_j.astype(np.int32)
    return nn_idx


def kernel(x, k):
    x = np.ascontiguousarray(np.asarray(x, dtype=np.float32))
    k = int(np.asarray(k))
    assert x.shape == (N, D) and k == 9, (x.shape, k)

    nc = _get_nc()
    in_maps = []
    for c in range(NCORES):
        if c == 0:
            xr = x
        else:
            xr = np.ascontiguousarray(
                np.concatenate([x[R * c:], x[:R * c]], axis=0))
        in_maps.append({"xin": xr})

    t0 = time.time()
    res = run_bass_kernel_spmd(nc, in_maps, core_ids=list(range(NCORES)))
    _CACHE["last_res"] = res
    _CACHE["last_exec_wall_s"] = time.time() - t0
    vals = np.stack([r["vals_out"] for r in res.results])   # [8, 1250, 32]
    idxs = np.stack([r["idx_out"] for r in res.results])    # [8, 1250, 32]

    nn_idx = _merge(vals, idxs, x)
    center = np.broadcast_to(np.arange(N, dtype=np.int32)[:, None], (N, 9))
    return np.stack([nn_idx, np.ascontiguousarray(center)], axis=0)


if __name__ == "__main__":
    build_nc()
    print("built OK")
